# revision 16
# baseline (speedup 1.0000x reference)
"""Trainium2 Bass kernel for nn_EvacPolicy (segment_reduce).

Data-parallel over 8 NeuronCores: nodes sharded at graph boundaries, MLP
weights replicated, per-graph segment mean computed locally per shard via a
prefix-scan + boundary-column gather, heads computed locally per shard
(row-wise independent), host concatenates per-core outputs.

Layout trick: every core places its local graph j in the SAME column range
[E[j-1], E[j]) of its node stream, where E = cumsum(max-over-cores graph
size). All gather offsets are therefore identical across cores, so one SPMD
program serves all 8 cores with offsets baked at trace time (the program is
rebuilt per kernel() call; nothing input-specific is hardcoded here).

All constants (weights/biases/recip/...) ride in one packed [128, W] blob =
one DMA = one semaphore lane; this walrus build allows only one sync-wait
command per PE instruction and eight per NoOp, so dependency fan-in must
stay small.
"""

import math
import os
import sys
from contextlib import ExitStack

try:
    import concourse  # noqa: F401  (already on path, e.g. axon site)
except ImportError:
    for _p in ("/opt/trn_rl_repo",):
        if _p not in sys.path and os.path.isdir(_p):
            sys.path.insert(0, _p)

import numpy as np

import concourse.bass as bass
import concourse.bacc as bacc
import concourse.tile as tile
import concourse.mybir as mybir
from concourse.bass_utils import run_bass_kernel_spmd

FP32 = mybir.dt.float32
GELU = mybir.ActivationFunctionType.Gelu
IDENT = mybir.ActivationFunctionType.Identity
ADD = mybir.AluOpType.add
SUB = mybir.AluOpType.subtract
MUL = mybir.AluOpType.mult

N_CORES = 8
UNIT = 1024          # node columns per pipeline unit (2 PSUM banks fp32)
MMN = 512            # max moving free dim per fp32 matmul


def _round_up(x, m):
    return (x + m - 1) // m * m


def const_layout(G_PAD):
    """(name, rows, cols) slices packed along the blob's free dim."""
    return [
        ("w1a", 8, 128), ("w2ph", 128, 128), ("w1i", 6, 128),
        ("w2i", 128, 128),
        ("b1a", 128, 1), ("b2a", 128, 1), ("b1i", 128, 1), ("b2i", 128, 1),
        ("pfa", 1, 128), ("pfb", 1, 64), ("ones", 1, 128),
        ("fc1w", 128, 256), ("fc1wb", 64, 256),
        ("fc1b0", 128, 1), ("fc1b1", 128, 1),
        ("fc2w0", 128, 128), ("fc2w1", 128, 128), ("fc2b", 128, 1),
        ("shgdw", 128, 2), ("shgdb", 2, 1),
        ("c1w", 128, 128), ("c1wb", 64, 128), ("c1b", 128, 1),
        ("c2w", 128, 64), ("c2b", 64, 1), ("c3w", 64, 1), ("c3b", 1, 1),
        ("recip", 1, G_PAD), ("npad", 1, G_PAD),
    ]


# ----------------------------------------------------------------------------
# device program
# ----------------------------------------------------------------------------

def build_program(NC_PAD, NH_PAD, G_PAD, GSPLIT, col_a, col_b_half,
                  scan_b_on_gpsimd=False):
    """col_a[j]: gather column (pedhaz stream) of local graph j's last node.
    col_b_half[j] = (half, col): gather column in the inf stream."""
    nc = bacc.Bacc("TRN2", target_bir_lowering=False, debug=False,
                   num_devices=N_CORES)

    layout = const_layout(G_PAD)
    WCOLS = sum(c for _, _, c in layout)

    xph = nc.dram_tensor("xph", [8, NC_PAD], FP32, kind="ExternalInput")
    xinf = nc.dram_tensor("xinf", [6, NH_PAD], FP32, kind="ExternalInput")
    wblob_d = nc.dram_tensor("wblob", [128, WCOLS], FP32,
                             kind="ExternalInput")
    o_sg = nc.dram_tensor("o_sg", [2, G_PAD], FP32, kind="ExternalOutput")
    o_v = nc.dram_tensor("o_v", [1, G_PAD], FP32, kind="ExternalOutput")

    NU_A = NC_PAD // UNIT
    NU_B = NH_PAD // UNIT

    a_by_unit = [[] for _ in range(NU_A)]
    for j in range(G_PAD):
        a_by_unit[col_a[j] // UNIT].append(j)
    b_by_unit = [[] for _ in range(NU_B)]
    for j in range(G_PAD):
        h, c = col_b_half[j]
        b_by_unit[c // UNIT].append(j)

    with tile.TileContext(nc) as tc, ExitStack() as ctx:
        const = ctx.enter_context(tc.tile_pool(name="const", bufs=1))
        stage = ctx.enter_context(tc.tile_pool(name="stage", bufs=1))

        wblob = const.tile([128, WCOLS], FP32, name="wblob")
        nc.sync.dma_start(wblob[:, :], wblob_d[:, :])
        W = {}
        off = 0
        for name, rows, cols in layout:
            W[name] = wblob[0:rows, off:off + cols]
            off += cols

        zeros = const.tile([128, UNIT], FP32, name="zeros")
        nc.vector.memset(zeros[:, :], 0.0)

        # persistent staging (allocated before the unit pools open)
        ea = stage.tile([128, G_PAD + 1], FP32, name="ea")
        eb = stage.tile([128, G_PAD + 2], FP32, name="eb")
        suma = stage.tile([128, G_PAD], FP32, name="suma")
        nhi = G_PAD + 1 - GSPLIT
        ebs = stage.tile([64, nhi], FP32, name="ebs")
        sumb = stage.tile([64, G_PAD], FP32, name="sumb")
        nc.vector.memset(ea[:, 0:1], 0.0)
        nc.vector.memset(eb[0:64, 0:1], 0.0)
        nc.vector.memset(eb[64:128, GSPLIT + 1:GSPLIT + 2], 0.0)

        scan_b_eng = nc.gpsimd if scan_b_on_gpsimd else nc.vector

        # const blob + memsets land before any compute: keeps every PE
        # instruction at <=1 sync wait (walrus limit on this build).
        tc.strict_bb_all_engine_barrier()

        # ---------------- node pipeline ----------------
        with tc.tile_pool(name="xin", bufs=3) as xin, \
             tc.tile_pool(name="mid", bufs=3) as mid, \
             tc.tile_pool(name="scn", bufs=4) as scn, \
             tc.tile_pool(name="psum", bufs=2, space="PSUM") as psum:

            prev = {"a": None, "b": None}

            def pedhaz_unit(u):
                xt = xin.tile([8, UNIT], FP32, name="xt", tag="xt")
                nc.sync.dma_start(xt[:, :], xph[:, u * UNIT:(u + 1) * UNIT])
                ps1 = psum.tile([128, UNIT], FP32, name="ps1", tag="l1")
                for k in range(UNIT // MMN):
                    s = slice(k * MMN, (k + 1) * MMN)
                    nc.tensor.matmul(ps1[:, s], W["w1a"], xt[:, s],
                                     start=True, stop=True)
                h1 = mid.tile([128, UNIT], FP32, name="h1", tag="h1a")
                nc.scalar.activation(h1[:, :], ps1[:, :], GELU,
                                     bias=W["b1a"])
                ps2 = psum.tile([128, UNIT], FP32, name="ps2", tag="l2")
                for k in range(UNIT // MMN):
                    s = slice(k * MMN, (k + 1) * MMN)
                    nc.tensor.matmul(ps2[:, s], W["w2ph"], h1[:, s],
                                     start=True, stop=True)
                sa = scn.tile([128, UNIT], FP32, name="sa", tag="sa")
                nc.scalar.activation(sa[:, :], ps2[:, :], GELU,
                                     bias=W["b2a"])
                init = (0.0 if prev["a"] is None
                        else prev["a"][:, UNIT - 1:UNIT])
                nc.vector.tensor_tensor_scan(
                    sa[:, :], sa[:, :], zeros[:, :], init, ADD, ADD)
                prev["a"] = sa
                for j in a_by_unit[u]:
                    c = col_a[j] - u * UNIT
                    nc.vector.tensor_copy(ea[:, j + 1:j + 2], sa[:, c:c + 1])

            def inf_unit(u):
                xt = xin.tile([6, UNIT], FP32, name="xti", tag="xti")
                nc.sync.dma_start(xt[:, :], xinf[:, u * UNIT:(u + 1) * UNIT])
                ps1 = psum.tile([128, UNIT], FP32, name="ps1i", tag="l1")
                for k in range(UNIT // MMN):
                    s = slice(k * MMN, (k + 1) * MMN)
                    nc.tensor.matmul(ps1[:, s], W["w1i"], xt[:, s],
                                     start=True, stop=True)
                h1 = mid.tile([128, UNIT], FP32, name="h1i", tag="h1b")
                nc.scalar.activation(h1[:, :], ps1[:, :], GELU,
                                     bias=W["b1i"])
                ps2 = psum.tile([128, UNIT], FP32, name="ps2i", tag="l2")
                for k in range(UNIT // MMN):
                    s = slice(k * MMN, (k + 1) * MMN)
                    nc.tensor.matmul(ps2[:, s], W["w2i"], h1[:, s],
                                     start=True, stop=True)
                sb = scn.tile([128, UNIT], FP32, name="sb", tag="sb")
                nc.scalar.activation(sb[:, :], ps2[:, :], GELU,
                                     bias=W["b2i"])
                init = (0.0 if prev["b"] is None
                        else prev["b"][:, UNIT - 1:UNIT])
                scan_b_eng.tensor_tensor_scan(
                    sb[:, :], sb[:, :], zeros[:, :], init, ADD, ADD)
                prev["b"] = sb
                for j in b_by_unit[u]:
                    h, c = col_b_half[j]
                    c -= u * UNIT
                    p0 = 64 * h
                    dst = j + 1 if h == 0 else j + 2
                    nc.vector.tensor_copy(eb[p0:p0 + 64, dst:dst + 1],
                                          sb[p0:p0 + 64, c:c + 1])

            bi = 0
            for u in range(NU_A):
                pedhaz_unit(u)
                if u % 2 == 1 and bi < NU_B:
                    inf_unit(bi)
                    bi += 1
            while bi < NU_B:
                inf_unit(bi)
                bi += 1

        # ---------------- segment sums ----------------
        tc.strict_bb_all_engine_barrier()
        nc.vector.tensor_tensor(
            suma[:, :], ea[:, 1:G_PAD + 1], ea[:, 0:G_PAD], SUB)
        nc.sync.dma_start(ebs[:, :], eb[64:128, GSPLIT + 1:G_PAD + 2])
        if GSPLIT > 0:
            nc.vector.tensor_tensor(
                sumb[:, 0:GSPLIT], eb[0:64, 1:GSPLIT + 1],
                eb[0:64, 0:GSPLIT], SUB)
        nc.vector.tensor_tensor(
            sumb[:, GSPLIT:G_PAD], ebs[:, 1:nhi], ebs[:, 0:nhi - 1], SUB)

        # ---------------- heads ----------------
        tc.strict_bb_all_engine_barrier()
        with tc.tile_pool(name="hps", bufs=2, space="PSUM") as hps, \
             tc.tile_pool(name="hsb", bufs=2) as hsb:

            for g0 in range(0, G_PAD, MMN):
                gs = min(MMN, G_PAD - g0)
                gsl = slice(g0, g0 + gs)

                pr = hps.tile([128, gs], FP32, name="pr", tag="hp")
                nc.tensor.matmul(pr[:, :], W["ones"],
                                 W["recip"][:, gsl], start=True, stop=True)
                pc = hps.tile([128, gs], FP32, name="pc", tag="hp")
                nc.tensor.matmul(pc[:, :], W["pfa"], W["npad"][:, gsl],
                                 start=True, stop=True)

                gea = hsb.tile([128, gs], FP32, name="gea", tag="ga")
                nc.vector.tensor_tensor(gea[:, :], suma[:, gsl], pc[:, :],
                                        SUB)
                nc.vector.tensor_tensor(gea[:, :], gea[:, :], pr[:, :], MUL)

                pcb = hps.tile([64, gs], FP32, name="pcb", tag="hp")
                nc.tensor.matmul(pcb[:, :], W["pfb"], W["npad"][:, gsl],
                                 start=True, stop=True)
                geb = hsb.tile([64, gs], FP32, name="geb", tag="gb")
                nc.vector.tensor_tensor(geb[:, :], sumb[:, gsl], pcb[:, :],
                                        SUB)
                nc.vector.tensor_tensor(geb[:, :], geb[:, :], pr[0:64, :],
                                        MUL)

                def lin2(lhs_a, lhs_b, bias_t, act, m0, m1, name):
                    p = hps.tile([m1 - m0, gs], FP32, name="p" + name,
                                 tag="hp")
                    nc.tensor.matmul(p[:, :], lhs_a[:, m0:m1], gea[:, :],
                                     start=True, stop=False)
                    nc.tensor.matmul(p[:, :], lhs_b[:, m0:m1], geb[:, :],
                                     start=False, stop=True)
                    o = hsb.tile([m1 - m0, gs], FP32, name="s" + name,
                                 tag="hs" + name)
                    nc.scalar.activation(o[:, :], p[:, :], act, bias=bias_t)
                    return o

                d10 = lin2(W["fc1w"], W["fc1wb"], W["fc1b0"], GELU,
                           0, 128, "d10")
                d11 = lin2(W["fc1w"], W["fc1wb"], W["fc1b1"], GELU,
                           128, 256, "d11")

                pd2 = hps.tile([128, gs], FP32, name="pd2", tag="hp")
                nc.tensor.matmul(pd2[:, :], W["fc2w0"], d10[:, :],
                                 start=True, stop=False)
                nc.tensor.matmul(pd2[:, :], W["fc2w1"], d11[:, :],
                                 start=False, stop=True)
                d2 = hsb.tile([128, gs], FP32, name="d2", tag="d2")
                nc.scalar.activation(d2[:, :], pd2[:, :], GELU,
                                     bias=W["fc2b"])

                psg = hps.tile([2, gs], FP32, name="psg", tag="hp")
                nc.tensor.matmul(psg[:, :], W["shgdw"], d2[:, :],
                                 start=True, stop=True)
                sg = hsb.tile([2, gs], FP32, name="sg", tag="sg")
                nc.scalar.activation(sg[:, :], psg[:, :], IDENT,
                                     bias=W["shgdb"])
                nc.sync.dma_start(o_sg[:, gsl], sg[:, :])

                v1 = lin2(W["c1w"], W["c1wb"], W["c1b"], GELU, 0, 128, "v1")
                pv2 = hps.tile([64, gs], FP32, name="pv2", tag="hp")
                nc.tensor.matmul(pv2[:, :], W["c2w"], v1[:, :],
                                 start=True, stop=True)
                v2 = hsb.tile([64, gs], FP32, name="v2", tag="v2")
                nc.scalar.activation(v2[:, :], pv2[:, :], GELU,
                                     bias=W["c2b"])
                pv3 = hps.tile([1, gs], FP32, name="pv3", tag="hp")
                nc.tensor.matmul(pv3[:, :], W["c3w"], v2[:, :],
                                 start=True, stop=True)
                vo = hsb.tile([1, gs], FP32, name="vo", tag="vo")
                nc.scalar.activation(vo[:, :], pv3[:, :], IDENT,
                                     bias=W["c3b"])
                nc.sync.dma_start(o_v[:, gsl], vo[:, :])

    return nc


# ----------------------------------------------------------------------------
# host wrapper
# ----------------------------------------------------------------------------

_cache = {}


def _np_gelu(x):
    v = np.vectorize(math.erf)
    return 0.5 * x * (1.0 + v(x / math.sqrt(2.0)))


def _blockdiag(blocks, K, M):
    out = np.zeros((K, M), np.float32)
    for Wm, r, c in blocks:
        out[r:r + Wm.shape[0], c:c + Wm.shape[1]] = Wm
    return out


def kernel(x_ped, x_hazard, x_infra, batch, num_graphs,
           ped_W1, ped_b1, ped_W2, ped_b2,
           haz_W1, haz_b1, haz_W2, haz_b2,
           inf_W1, inf_b1, inf_W2, inf_b2,
           fc1_W, fc1_b, fc2_W, fc2_b,
           sh_W, sh_b, gd_W, gd_b,
           c1_W, c1_b, c2_W, c2_b, c3_W, c3_b):
    x_ped = np.asarray(x_ped, np.float32)
    x_hazard = np.asarray(x_hazard, np.float32)
    x_infra = np.asarray(x_infra, np.float32)
    batch = np.asarray(batch).astype(np.int64)
    B = int(num_graphs)
    N = batch.shape[0]

    f32 = lambda a: np.ascontiguousarray(np.asarray(a), dtype=np.float32)
    (ped_W1, ped_b1, ped_W2, ped_b2, haz_W1, haz_b1, haz_W2, haz_b2,
     inf_W1, inf_b1, inf_W2, inf_b2, fc1_W, fc1_b, fc2_W, fc2_b,
     sh_W, sh_b, gd_W, gd_b, c1_W, c1_b, c2_W, c2_b, c3_W, c3_b) = map(
        f32, (ped_W1, ped_b1, ped_W2, ped_b2, haz_W1, haz_b1, haz_W2,
              haz_b2, inf_W1, inf_b1, inf_W2, inf_b2, fc1_W, fc1_b, fc2_W,
              fc2_b, sh_W, sh_b, gd_W, gd_b, c1_W, c1_b, c2_W, c2_b, c3_W,
              c3_b))

    # ---- shard graphs across cores, balancing node counts ----
    ends = np.searchsorted(batch, np.arange(B), side="right")
    gsplits = [0]
    for c in range(1, N_CORES):
        gsplits.append(int(np.searchsorted(ends, N * c // N_CORES)))
    gsplits.append(B)
    g_lo = np.array(gsplits[:-1])
    g_hi = np.maximum(np.array(gsplits[1:]), g_lo)

    counts_all = np.diff(np.concatenate([[0], ends])).astype(np.int64)
    G_PAD = _round_up(max(int((g_hi - g_lo).max()), 2), 64)

    cnt = np.zeros((N_CORES, G_PAD), np.int64)
    for c in range(N_CORES):
        g = g_hi[c] - g_lo[c]
        cnt[c, :g] = counts_all[g_lo[c]:g_hi[c]]

    maxcnt = np.maximum(cnt.max(axis=0), 1)
    E = np.cumsum(maxcnt)
    NC_REQ = int(E[-1])
    NC_PAD = _round_up(NC_REQ, UNIT)
    GSPLIT = G_PAD // 2
    NH0 = int(E[GSPLIT - 1])
    NH_PAD = _round_up(max(NH0, NC_REQ - NH0), UNIT)

    col_a = [int(E[j] - 1) for j in range(G_PAD)]
    col_b_half = [(0, int(E[j] - 1)) if j < GSPLIT
                  else (1, int(E[j] - 1 - NH0)) for j in range(G_PAD)]

    # ---- per-core node arrays ----
    E_prev = np.concatenate([[0], E[:-1]])
    starts_all = ends - counts_all
    x_all = np.concatenate([x_ped, x_hazard, x_infra], axis=1)

    in_maps = []
    for c in range(N_CORES):
        n0 = int(ends[g_lo[c] - 1]) if g_lo[c] > 0 else 0
        n1 = int(ends[g_hi[c] - 1]) if g_hi[c] > 0 else 0
        ncr = n1 - n0
        g = batch[n0:n1] - g_lo[c]
        dest = E_prev[g] + (np.arange(ncr) - (starts_all[batch[n0:n1]] - n0))

        xph_a = np.zeros((8, NC_PAD), np.float32)
        xph_a[:, dest] = x_all[n0:n1].T
        xinf_a = np.zeros((6, NH_PAD), np.float32)
        lo = dest < NH0
        xinf_a[0:3, dest[lo]] = x_infra[n0:n1][lo].T
        xinf_a[3:6, dest[~lo] - NH0] = x_infra[n0:n1][~lo].T
        in_maps.append({"xph": xph_a, "xinf": xinf_a})

    # feature vector produced by zero-input (padding) nodes
    pf = []
    for W1, b1, W2, b2 in ((ped_W1, ped_b1, ped_W2, ped_b2),
                           (haz_W1, haz_b1, haz_W2, haz_b2),
                           (inf_W1, inf_b1, inf_W2, inf_b2)):
        pf.append(_np_gelu(_np_gelu(b1) @ W2 + b2))
    pad_feat = np.concatenate(pf).astype(np.float32)

    consts = {
        "w1a": _blockdiag([(ped_W1, 0, 0), (haz_W1, 2, 64)], 8, 128),
        "w2ph": _blockdiag([(ped_W2, 0, 0), (haz_W2, 64, 64)], 128, 128),
        "w1i": _blockdiag([(inf_W1, 0, 0), (inf_W1, 3, 64)], 6, 128),
        "w2i": _blockdiag([(inf_W2, 0, 0), (inf_W2, 64, 64)], 128, 128),
        "b1a": np.concatenate([ped_b1, haz_b1]).reshape(128, 1),
        "b2a": np.concatenate([ped_b2, haz_b2]).reshape(128, 1),
        "b1i": np.concatenate([inf_b1, inf_b1]).reshape(128, 1),
        "b2i": np.concatenate([inf_b2, inf_b2]).reshape(128, 1),
        "pfa": pad_feat[:128].reshape(1, 128),
        "pfb": pad_feat[128:].reshape(1, 64),
        "ones": np.ones((1, 128), np.float32),
        "fc1w": fc1_W[0:128], "fc1wb": fc1_W[128:192],
        "fc1b0": fc1_b[0:128].reshape(128, 1),
        "fc1b1": fc1_b[128:256].reshape(128, 1),
        "fc2w0": fc2_W[0:128], "fc2w1": fc2_W[128:256],
        "fc2b": fc2_b.reshape(128, 1),
        "shgdw": np.concatenate([sh_W, gd_W], axis=1),
        "shgdb": np.array([[float(sh_b.ravel()[0])],
                           [float(gd_b.ravel()[0])]], np.float32),
        "c1w": c1_W[0:128], "c1wb": c1_W[128:192],
        "c1b": c1_b.reshape(128, 1),
        "c2w": c2_W, "c2b": c2_b.reshape(64, 1),
        "c3w": c3_W, "c3b": c3_b.reshape(1, 1),
    }

    layout = const_layout(G_PAD)
    WCOLS = sum(c for _, _, c in layout)
    blob_common = np.zeros((128, WCOLS), np.float32)
    slices = {}
    off = 0
    for name, rows, cols in layout:
        slices[name] = (rows, slice(off, off + cols))
        if name in consts:
            blob_common[0:rows, off:off + cols] = consts[name]
        off += cols

    for c in range(N_CORES):
        blob = blob_common.copy()
        rows, sl = slices["recip"]
        blob[0:rows, sl] = (1.0 / np.maximum(cnt[c], 1)).astype(np.float32)
        rows, sl = slices["npad"]
        blob[0:rows, sl] = (maxcnt - cnt[c]).astype(np.float32)
        in_maps[c]["wblob"] = blob

    # ---- build / fetch program ----
    key = (NC_PAD, NH_PAD, G_PAD, GSPLIT, tuple(col_a), tuple(col_b_half))
    if key not in _cache:
        _cache.clear()
        nc_new = build_program(NC_PAD, NH_PAD, G_PAD, GSPLIT,
                               col_a, col_b_half)
        if not nc_new.is_finalized():
            nc_new.finalize()
        _cache[key] = nc_new
    nc = _cache[key]

    trace = bool(os.environ.get("BASS_PROFILE"))
    if trace:
        _ensure_ntff_hook()
    res = run_bass_kernel_spmd(nc, in_maps, list(range(N_CORES)),
                               trace=trace)
    if trace and res.exec_time_ns is not None:
        kernel.last_exec_time_ns = res.exec_time_ns
        kernel.last_result = res
        print(f"HW exec time: {res.exec_time_ns} ns")

    # ---- assemble full outputs ----
    shelter = np.zeros((B, 1), np.float32)
    guidance = np.zeros((B, 1), np.float32)
    value = np.zeros((B,), np.float32)
    for c in range(N_CORES):
        g = g_hi[c] - g_lo[c]
        if g == 0:
            continue
        out_sg = res.results[c]["o_sg"]
        out_v = res.results[c]["o_v"]
        shelter[g_lo[c]:g_hi[c], 0] = out_sg[0, :g]
        guidance[g_lo[c]:g_hi[c], 0] = out_sg[1, :g]
        value[g_lo[c]:g_hi[c]] = out_v[0, :g]
    return shelter, guidance, value


kernel.last_exec_time_ns = None
kernel.last_result = None


def _ensure_ntff_hook():
    """The agent image lacks ``antenv.axon_hooks``; shim it with the
    ctypes NTFF profiler from trn_agent_boot so trace=True works."""
    import types
    try:
        from antenv.axon_hooks import get_axon_ntff_profile_hook  # noqa
        return
    except ImportError:
        pass
    try:
        sys.path.insert(0, "/root/.axon_site")
        from trn_agent_boot.trn_boot import _ntff_profile_via_ctypes
        hook = _ntff_profile_via_ctypes("/opt/axon/libaxon_pjrt.so")
    except Exception:
        hook = None
    store = {"h": hook}
    pkg = sys.modules.setdefault("antenv", types.ModuleType("antenv"))
    mod = types.ModuleType("antenv.axon_hooks")
    mod.get_axon_ntff_profile_hook = lambda: store["h"]
    mod.set_axon_ntff_profile_hook = lambda h: store.update(h=h)
    pkg.axon_hooks = mod
    sys.modules["antenv.axon_hooks"] = mod


# revision 22
# speedup vs baseline: 1.5846x; 1.5846x over previous
"""Trainium2 Bass kernel for nn_EvacPolicy (segment_reduce).

Data-parallel over 8 NeuronCores: nodes sharded at graph boundaries, MLP
weights replicated, per-graph segment mean computed locally per shard via a
prefix-scan + boundary-column gather, heads computed locally per shard
(row-wise independent), host concatenates per-core outputs.

Layout trick: every core places its local graph j in the SAME column range
[E[j-1], E[j]) of its node stream, where E = cumsum(max-over-cores graph
size). All gather offsets are therefore identical across cores, so one SPMD
program serves all 8 cores with offsets baked at trace time (the program is
rebuilt per kernel() call; nothing input-specific is hardcoded here).

All constants (weights/biases/recip/...) ride in one packed [128, W] blob =
one DMA = one semaphore lane; this walrus build allows only one sync-wait
command per PE instruction and eight per NoOp, so dependency fan-in must
stay small.
"""

import math
import os
import sys
from contextlib import ExitStack

try:
    import concourse  # noqa: F401  (already on path, e.g. axon site)
except ImportError:
    for _p in ("/opt/trn_rl_repo",):
        if _p not in sys.path and os.path.isdir(_p):
            sys.path.insert(0, _p)

import numpy as np

import concourse.bass as bass
import concourse.bacc as bacc
import concourse.tile as tile
import concourse.mybir as mybir
from concourse.bass_utils import run_bass_kernel_spmd

FP32 = mybir.dt.float32
BF16 = mybir.dt.bfloat16
GELU = mybir.ActivationFunctionType.Gelu
IDENT = mybir.ActivationFunctionType.Identity
ADD = mybir.AluOpType.add
SUB = mybir.AluOpType.subtract
MUL = mybir.AluOpType.mult

N_CORES = 8
UNIT = 1024          # node columns per pipeline unit (2 PSUM banks fp32)
MMN = 512            # max moving free dim per fp32 matmul


def _round_up(x, m):
    return (x + m - 1) // m * m


def const_layout(G_PAD):
    """(name, rows, cols) slices packed along the blob's free dim."""
    return [
        ("w1a", 8, 128), ("w2ph", 128, 128), ("w1i", 6, 128),
        ("w2i", 128, 128),
        ("b1a", 128, 1), ("b2a", 128, 1), ("b1i", 128, 1), ("b2i", 128, 1),
        ("pfa", 1, 128), ("pfb", 1, 64), ("ones", 1, 128),
        ("fc1w", 128, 256), ("fc1wb", 64, 256),
        ("fc1b0", 128, 1), ("fc1b1", 128, 1),
        ("fc2w0", 128, 128), ("fc2w1", 128, 128), ("fc2b", 128, 1),
        ("shgdw", 128, 2), ("shgdb", 2, 1),
        ("c1w", 128, 128), ("c1wb", 64, 128), ("c1b", 128, 1),
        ("c2w", 128, 64), ("c2b", 64, 1), ("c3w", 64, 1), ("c3b", 1, 1),
        ("recip", 1, G_PAD), ("npad", 1, G_PAD),
    ]


# ----------------------------------------------------------------------------
# device program
# ----------------------------------------------------------------------------

def build_program(NC_PAD, NH_PAD, G_PAD, GSPLIT, col_a, col_b_half,
                  scan_b_on_gpsimd=False):
    """col_a[j]: gather column (pedhaz stream) of local graph j's last node.
    col_b_half[j] = (half, col): gather column in the inf stream."""
    nc = bacc.Bacc("TRN2", target_bir_lowering=False, debug=False,
                   num_devices=N_CORES)

    layout = const_layout(G_PAD)
    WCOLS = sum(c for _, _, c in layout)

    xph = nc.dram_tensor("xph", [8, NC_PAD], BF16, kind="ExternalInput")
    xinf = nc.dram_tensor("xinf", [6, NH_PAD], BF16, kind="ExternalInput")
    wblob_d = nc.dram_tensor("wblob", [128, WCOLS], FP32,
                             kind="ExternalInput")
    wb16_d = nc.dram_tensor("wb16", [128, 512], BF16, kind="ExternalInput")
    o_sg = nc.dram_tensor("o_sg", [2, G_PAD], FP32, kind="ExternalOutput")
    o_v = nc.dram_tensor("o_v", [1, G_PAD], FP32, kind="ExternalOutput")

    NU_A = NC_PAD // UNIT
    NU_B = NH_PAD // UNIT

    a_by_unit = [[] for _ in range(NU_A)]
    for j in range(G_PAD):
        a_by_unit[col_a[j] // UNIT].append(j)
    b_by_unit = [[] for _ in range(NU_B)]
    for j in range(G_PAD):
        h, c = col_b_half[j]
        b_by_unit[c // UNIT].append(j)

    with tile.TileContext(nc) as tc, ExitStack() as ctx:
        const = ctx.enter_context(tc.tile_pool(name="const", bufs=1))
        stage = ctx.enter_context(tc.tile_pool(name="stage", bufs=1))

        wblob = const.tile([128, WCOLS], FP32, name="wblob")
        nc.sync.dma_start(wblob[:, :], wblob_d[:, :])
        W = {}
        off = 0
        for name, rows, cols in layout:
            W[name] = wblob[0:rows, off:off + cols]
            off += cols
        # bf16 copies of the node-MLP weights (single-pass matmuls + FWL)
        wb16 = const.tile([128, 512], BF16, name="wb16")
        nc.sync.dma_start(wb16[:, :], wb16_d[:, :])
        W["w1a"] = wb16[0:8, 0:128]
        W["w2ph"] = wb16[0:128, 128:256]
        W["w1i"] = wb16[0:6, 256:384]
        W["w2i"] = wb16[0:128, 384:512]

        zeros = const.tile([128, UNIT], FP32, name="zeros")
        nc.vector.memset(zeros[:, :], 0.0)

        # persistent staging (allocated before the unit pools open)
        ea = stage.tile([128, G_PAD + 1], FP32, name="ea")
        eb = stage.tile([128, G_PAD + 2], FP32, name="eb")
        suma = stage.tile([128, G_PAD], FP32, name="suma")
        nhi = G_PAD + 1 - GSPLIT
        ebs = stage.tile([64, nhi], FP32, name="ebs")
        sumb = stage.tile([64, G_PAD], FP32, name="sumb")
        nc.vector.memset(ea[:, 0:1], 0.0)
        nc.vector.memset(eb[0:64, 0:1], 0.0)
        nc.vector.memset(eb[64:128, GSPLIT + 1:GSPLIT + 2], 0.0)

        scan_b_eng = nc.gpsimd if scan_b_on_gpsimd else nc.vector

        # const blob + memsets land before any compute: keeps every PE
        # instruction at <=1 sync wait (walrus limit on this build).
        tc.strict_bb_all_engine_barrier()

        # ---------------- node pipeline ----------------
        with tc.tile_pool(name="xin", bufs=3) as xin, \
             tc.tile_pool(name="mid", bufs=3) as mid, \
             tc.tile_pool(name="scn", bufs=4) as scn, \
             tc.tile_pool(name="psum", bufs=2, space="PSUM") as psum:

            prev = {"a": None, "b": None}

            def pedhaz_unit(u):
                xt = xin.tile([8, UNIT], BF16, name="xt", tag="xt")
                nc.sync.dma_start(xt[:, :], xph[:, u * UNIT:(u + 1) * UNIT])
                ps1 = psum.tile([128, UNIT], FP32, name="ps1", tag="l1")
                for k in range(UNIT // MMN):
                    s = slice(k * MMN, (k + 1) * MMN)
                    nc.tensor.matmul(ps1[:, s], W["w1a"], xt[:, s],
                                     start=True, stop=True)
                h1 = mid.tile([128, UNIT], BF16, name="h1", tag="h1a")
                nc.scalar.activation(h1[:, :], ps1[:, :], GELU,
                                     bias=W["b1a"])
                ps2 = psum.tile([128, UNIT], FP32, name="ps2", tag="l2")
                for k in range(UNIT // MMN):
                    s = slice(k * MMN, (k + 1) * MMN)
                    nc.tensor.matmul(ps2[:, s], W["w2ph"], h1[:, s],
                                     start=True, stop=True)
                sa = scn.tile([128, UNIT], FP32, name="sa", tag="sa")
                nc.scalar.activation(sa[:, :], ps2[:, :], GELU,
                                     bias=W["b2a"])
                init = (0.0 if prev["a"] is None
                        else prev["a"][:, UNIT - 1:UNIT])
                nc.vector.tensor_tensor_scan(
                    sa[:, :], sa[:, :], zeros[:, :], init, ADD, ADD)
                prev["a"] = sa
                for j in a_by_unit[u]:
                    c = col_a[j] - u * UNIT
                    nc.vector.tensor_copy(ea[:, j + 1:j + 2], sa[:, c:c + 1])

            def inf_unit(u):
                xt = xin.tile([6, UNIT], BF16, name="xti", tag="xti")
                nc.sync.dma_start(xt[:, :], xinf[:, u * UNIT:(u + 1) * UNIT])
                ps1 = psum.tile([128, UNIT], FP32, name="ps1i", tag="l1")
                for k in range(UNIT // MMN):
                    s = slice(k * MMN, (k + 1) * MMN)
                    nc.tensor.matmul(ps1[:, s], W["w1i"], xt[:, s],
                                     start=True, stop=True)
                h1 = mid.tile([128, UNIT], BF16, name="h1i", tag="h1b")
                nc.scalar.activation(h1[:, :], ps1[:, :], GELU,
                                     bias=W["b1i"])
                ps2 = psum.tile([128, UNIT], FP32, name="ps2i", tag="l2")
                for k in range(UNIT // MMN):
                    s = slice(k * MMN, (k + 1) * MMN)
                    nc.tensor.matmul(ps2[:, s], W["w2i"], h1[:, s],
                                     start=True, stop=True)
                sb = scn.tile([128, UNIT], FP32, name="sb", tag="sb")
                nc.scalar.activation(sb[:, :], ps2[:, :], GELU,
                                     bias=W["b2i"])
                init = (0.0 if prev["b"] is None
                        else prev["b"][:, UNIT - 1:UNIT])
                scan_b_eng.tensor_tensor_scan(
                    sb[:, :], sb[:, :], zeros[:, :], init, ADD, ADD)
                prev["b"] = sb
                for j in b_by_unit[u]:
                    h, c = col_b_half[j]
                    c -= u * UNIT
                    p0 = 64 * h
                    dst = j + 1 if h == 0 else j + 2
                    nc.vector.tensor_copy(eb[p0:p0 + 64, dst:dst + 1],
                                          sb[p0:p0 + 64, c:c + 1])

            bi = 0
            for u in range(NU_A):
                pedhaz_unit(u)
                if u % 2 == 1 and bi < NU_B:
                    inf_unit(bi)
                    bi += 1
            while bi < NU_B:
                inf_unit(bi)
                bi += 1

        # ---------------- segment sums ----------------
        tc.strict_bb_all_engine_barrier()
        nc.vector.tensor_tensor(
            suma[:, :], ea[:, 1:G_PAD + 1], ea[:, 0:G_PAD], SUB)
        nc.sync.dma_start(ebs[:, :], eb[64:128, GSPLIT + 1:G_PAD + 2])
        if GSPLIT > 0:
            nc.vector.tensor_tensor(
                sumb[:, 0:GSPLIT], eb[0:64, 1:GSPLIT + 1],
                eb[0:64, 0:GSPLIT], SUB)
        nc.vector.tensor_tensor(
            sumb[:, GSPLIT:G_PAD], ebs[:, 1:nhi], ebs[:, 0:nhi - 1], SUB)

        # ---------------- heads ----------------
        tc.strict_bb_all_engine_barrier()
        with tc.tile_pool(name="hps", bufs=2, space="PSUM") as hps, \
             tc.tile_pool(name="hsb", bufs=2) as hsb:

            for g0 in range(0, G_PAD, MMN):
                gs = min(MMN, G_PAD - g0)
                gsl = slice(g0, g0 + gs)

                pr = hps.tile([128, gs], FP32, name="pr", tag="hp")
                nc.tensor.matmul(pr[:, :], W["ones"],
                                 W["recip"][:, gsl], start=True, stop=True)
                pc = hps.tile([128, gs], FP32, name="pc", tag="hp")
                nc.tensor.matmul(pc[:, :], W["pfa"], W["npad"][:, gsl],
                                 start=True, stop=True)

                gea = hsb.tile([128, gs], FP32, name="gea", tag="ga")
                nc.vector.tensor_tensor(gea[:, :], suma[:, gsl], pc[:, :],
                                        SUB)
                nc.vector.tensor_tensor(gea[:, :], gea[:, :], pr[:, :], MUL)

                pcb = hps.tile([64, gs], FP32, name="pcb", tag="hp")
                nc.tensor.matmul(pcb[:, :], W["pfb"], W["npad"][:, gsl],
                                 start=True, stop=True)
                geb = hsb.tile([64, gs], FP32, name="geb", tag="gb")
                nc.vector.tensor_tensor(geb[:, :], sumb[:, gsl], pcb[:, :],
                                        SUB)
                nc.vector.tensor_tensor(geb[:, :], geb[:, :], pr[0:64, :],
                                        MUL)

                def lin2(lhs_a, lhs_b, bias_t, act, m0, m1, name):
                    p = hps.tile([m1 - m0, gs], FP32, name="p" + name,
                                 tag="hp")
                    nc.tensor.matmul(p[:, :], lhs_a[:, m0:m1], gea[:, :],
                                     start=True, stop=False)
                    nc.tensor.matmul(p[:, :], lhs_b[:, m0:m1], geb[:, :],
                                     start=False, stop=True)
                    o = hsb.tile([m1 - m0, gs], FP32, name="s" + name,
                                 tag="hs" + name)
                    nc.scalar.activation(o[:, :], p[:, :], act, bias=bias_t)
                    return o

                d10 = lin2(W["fc1w"], W["fc1wb"], W["fc1b0"], GELU,
                           0, 128, "d10")
                d11 = lin2(W["fc1w"], W["fc1wb"], W["fc1b1"], GELU,
                           128, 256, "d11")

                pd2 = hps.tile([128, gs], FP32, name="pd2", tag="hp")
                nc.tensor.matmul(pd2[:, :], W["fc2w0"], d10[:, :],
                                 start=True, stop=False)
                nc.tensor.matmul(pd2[:, :], W["fc2w1"], d11[:, :],
                                 start=False, stop=True)
                d2 = hsb.tile([128, gs], FP32, name="d2", tag="d2")
                nc.scalar.activation(d2[:, :], pd2[:, :], GELU,
                                     bias=W["fc2b"])

                psg = hps.tile([2, gs], FP32, name="psg", tag="hp")
                nc.tensor.matmul(psg[:, :], W["shgdw"], d2[:, :],
                                 start=True, stop=True)
                sg = hsb.tile([2, gs], FP32, name="sg", tag="sg")
                nc.scalar.activation(sg[:, :], psg[:, :], IDENT,
                                     bias=W["shgdb"])
                nc.sync.dma_start(o_sg[:, gsl], sg[:, :])

                v1 = lin2(W["c1w"], W["c1wb"], W["c1b"], GELU, 0, 128, "v1")
                pv2 = hps.tile([64, gs], FP32, name="pv2", tag="hp")
                nc.tensor.matmul(pv2[:, :], W["c2w"], v1[:, :],
                                 start=True, stop=True)
                v2 = hsb.tile([64, gs], FP32, name="v2", tag="v2")
                nc.scalar.activation(v2[:, :], pv2[:, :], GELU,
                                     bias=W["c2b"])
                pv3 = hps.tile([1, gs], FP32, name="pv3", tag="hp")
                nc.tensor.matmul(pv3[:, :], W["c3w"], v2[:, :],
                                 start=True, stop=True)
                vo = hsb.tile([1, gs], FP32, name="vo", tag="vo")
                nc.scalar.activation(vo[:, :], pv3[:, :], IDENT,
                                     bias=W["c3b"])
                nc.sync.dma_start(o_v[:, gsl], vo[:, :])

    return nc


# ----------------------------------------------------------------------------
# host wrapper
# ----------------------------------------------------------------------------

_cache = {}


def _np_gelu(x):
    v = np.vectorize(math.erf)
    return 0.5 * x * (1.0 + v(x / math.sqrt(2.0)))


def _blockdiag(blocks, K, M):
    out = np.zeros((K, M), np.float32)
    for Wm, r, c in blocks:
        out[r:r + Wm.shape[0], c:c + Wm.shape[1]] = Wm
    return out


def kernel(x_ped, x_hazard, x_infra, batch, num_graphs,
           ped_W1, ped_b1, ped_W2, ped_b2,
           haz_W1, haz_b1, haz_W2, haz_b2,
           inf_W1, inf_b1, inf_W2, inf_b2,
           fc1_W, fc1_b, fc2_W, fc2_b,
           sh_W, sh_b, gd_W, gd_b,
           c1_W, c1_b, c2_W, c2_b, c3_W, c3_b):
    x_ped = np.asarray(x_ped, np.float32)
    x_hazard = np.asarray(x_hazard, np.float32)
    x_infra = np.asarray(x_infra, np.float32)
    batch = np.asarray(batch).astype(np.int64)
    B = int(num_graphs)
    N = batch.shape[0]

    f32 = lambda a: np.ascontiguousarray(np.asarray(a), dtype=np.float32)
    (ped_W1, ped_b1, ped_W2, ped_b2, haz_W1, haz_b1, haz_W2, haz_b2,
     inf_W1, inf_b1, inf_W2, inf_b2, fc1_W, fc1_b, fc2_W, fc2_b,
     sh_W, sh_b, gd_W, gd_b, c1_W, c1_b, c2_W, c2_b, c3_W, c3_b) = map(
        f32, (ped_W1, ped_b1, ped_W2, ped_b2, haz_W1, haz_b1, haz_W2,
              haz_b2, inf_W1, inf_b1, inf_W2, inf_b2, fc1_W, fc1_b, fc2_W,
              fc2_b, sh_W, sh_b, gd_W, gd_b, c1_W, c1_b, c2_W, c2_b, c3_W,
              c3_b))

    # ---- shard graphs across cores, balancing node counts ----
    ends = np.searchsorted(batch, np.arange(B), side="right")
    gsplits = [0]
    for c in range(1, N_CORES):
        gsplits.append(int(np.searchsorted(ends, N * c // N_CORES)))
    gsplits.append(B)
    g_lo = np.array(gsplits[:-1])
    g_hi = np.maximum(np.array(gsplits[1:]), g_lo)

    counts_all = np.diff(np.concatenate([[0], ends])).astype(np.int64)
    G_PAD = _round_up(max(int((g_hi - g_lo).max()), 2), 64)

    cnt = np.zeros((N_CORES, G_PAD), np.int64)
    for c in range(N_CORES):
        g = g_hi[c] - g_lo[c]
        cnt[c, :g] = counts_all[g_lo[c]:g_hi[c]]

    maxcnt = np.maximum(cnt.max(axis=0), 1)
    E = np.cumsum(maxcnt)
    NC_REQ = int(E[-1])
    NC_PAD = _round_up(NC_REQ, UNIT)
    GSPLIT = G_PAD // 2
    NH0 = int(E[GSPLIT - 1])
    NH_PAD = _round_up(max(NH0, NC_REQ - NH0), UNIT)

    col_a = [int(E[j] - 1) for j in range(G_PAD)]
    col_b_half = [(0, int(E[j] - 1)) if j < GSPLIT
                  else (1, int(E[j] - 1 - NH0)) for j in range(G_PAD)]

    # ---- per-core node arrays ----
    E_prev = np.concatenate([[0], E[:-1]])
    starts_all = ends - counts_all
    x_all = np.concatenate([x_ped, x_hazard, x_infra], axis=1)

    in_maps = []
    for c in range(N_CORES):
        n0 = int(ends[g_lo[c] - 1]) if g_lo[c] > 0 else 0
        n1 = int(ends[g_hi[c] - 1]) if g_hi[c] > 0 else 0
        ncr = n1 - n0
        g = batch[n0:n1] - g_lo[c]
        dest = E_prev[g] + (np.arange(ncr) - (starts_all[batch[n0:n1]] - n0))

        import ml_dtypes
        bf16 = ml_dtypes.bfloat16
        xph_a = np.zeros((8, NC_PAD), bf16)
        xph_a[:, dest] = x_all[n0:n1].T.astype(bf16)
        xinf_a = np.zeros((6, NH_PAD), bf16)
        lo = dest < NH0
        xinf_a[0:3, dest[lo]] = x_infra[n0:n1][lo].T.astype(bf16)
        xinf_a[3:6, dest[~lo] - NH0] = x_infra[n0:n1][~lo].T.astype(bf16)
        in_maps.append({"xph": xph_a, "xinf": xinf_a})

    # feature vector produced by zero-input (padding) nodes
    pf = []
    for W1, b1, W2, b2 in ((ped_W1, ped_b1, ped_W2, ped_b2),
                           (haz_W1, haz_b1, haz_W2, haz_b2),
                           (inf_W1, inf_b1, inf_W2, inf_b2)):
        pf.append(_np_gelu(_np_gelu(b1) @ W2 + b2))
    pad_feat = np.concatenate(pf).astype(np.float32)

    consts = {
        "w1a": _blockdiag([(ped_W1, 0, 0), (haz_W1, 2, 64)], 8, 128),
        "w2ph": _blockdiag([(ped_W2, 0, 0), (haz_W2, 64, 64)], 128, 128),
        "w1i": _blockdiag([(inf_W1, 0, 0), (inf_W1, 3, 64)], 6, 128),
        "w2i": _blockdiag([(inf_W2, 0, 0), (inf_W2, 64, 64)], 128, 128),
        "b1a": np.concatenate([ped_b1, haz_b1]).reshape(128, 1),
        "b2a": np.concatenate([ped_b2, haz_b2]).reshape(128, 1),
        "b1i": np.concatenate([inf_b1, inf_b1]).reshape(128, 1),
        "b2i": np.concatenate([inf_b2, inf_b2]).reshape(128, 1),
        "pfa": pad_feat[:128].reshape(1, 128),
        "pfb": pad_feat[128:].reshape(1, 64),
        "ones": np.ones((1, 128), np.float32),
        "fc1w": fc1_W[0:128], "fc1wb": fc1_W[128:192],
        "fc1b0": fc1_b[0:128].reshape(128, 1),
        "fc1b1": fc1_b[128:256].reshape(128, 1),
        "fc2w0": fc2_W[0:128], "fc2w1": fc2_W[128:256],
        "fc2b": fc2_b.reshape(128, 1),
        "shgdw": np.concatenate([sh_W, gd_W], axis=1),
        "shgdb": np.array([[float(sh_b.ravel()[0])],
                           [float(gd_b.ravel()[0])]], np.float32),
        "c1w": c1_W[0:128], "c1wb": c1_W[128:192],
        "c1b": c1_b.reshape(128, 1),
        "c2w": c2_W, "c2b": c2_b.reshape(64, 1),
        "c3w": c3_W, "c3b": c3_b.reshape(1, 1),
    }

    layout = const_layout(G_PAD)
    WCOLS = sum(c for _, _, c in layout)
    blob_common = np.zeros((128, WCOLS), np.float32)
    slices = {}
    off = 0
    for name, rows, cols in layout:
        slices[name] = (rows, slice(off, off + cols))
        if name in consts:
            blob_common[0:rows, off:off + cols] = consts[name]
        off += cols

    import ml_dtypes
    wb16 = np.zeros((128, 512), ml_dtypes.bfloat16)
    wb16[0:8, 0:128] = consts["w1a"].astype(ml_dtypes.bfloat16)
    wb16[0:128, 128:256] = consts["w2ph"].astype(ml_dtypes.bfloat16)
    wb16[0:6, 256:384] = consts["w1i"].astype(ml_dtypes.bfloat16)
    wb16[0:128, 384:512] = consts["w2i"].astype(ml_dtypes.bfloat16)

    for c in range(N_CORES):
        blob = blob_common.copy()
        rows, sl = slices["recip"]
        blob[0:rows, sl] = (1.0 / np.maximum(cnt[c], 1)).astype(np.float32)
        rows, sl = slices["npad"]
        blob[0:rows, sl] = (maxcnt - cnt[c]).astype(np.float32)
        in_maps[c]["wblob"] = blob
        in_maps[c]["wb16"] = wb16

    # ---- build / fetch program ----
    key = (NC_PAD, NH_PAD, G_PAD, GSPLIT, tuple(col_a), tuple(col_b_half))
    if key not in _cache:
        _cache.clear()
        nc_new = build_program(NC_PAD, NH_PAD, G_PAD, GSPLIT,
                               col_a, col_b_half)
        if not nc_new.is_finalized():
            nc_new.finalize()
        _cache[key] = nc_new
    nc = _cache[key]

    trace = bool(os.environ.get("BASS_PROFILE"))
    if trace:
        _ensure_ntff_hook()
    res = run_bass_kernel_spmd(nc, in_maps, list(range(N_CORES)),
                               trace=trace)
    if trace and res.exec_time_ns is not None:
        kernel.last_exec_time_ns = res.exec_time_ns
        kernel.last_result = res
        print(f"HW exec time: {res.exec_time_ns} ns")

    # ---- assemble full outputs ----
    shelter = np.zeros((B, 1), np.float32)
    guidance = np.zeros((B, 1), np.float32)
    value = np.zeros((B,), np.float32)
    for c in range(N_CORES):
        g = g_hi[c] - g_lo[c]
        if g == 0:
            continue
        out_sg = res.results[c]["o_sg"]
        out_v = res.results[c]["o_v"]
        shelter[g_lo[c]:g_hi[c], 0] = out_sg[0, :g]
        guidance[g_lo[c]:g_hi[c], 0] = out_sg[1, :g]
        value[g_lo[c]:g_hi[c]] = out_v[0, :g]
    return shelter, guidance, value


kernel.last_exec_time_ns = None
kernel.last_result = None


def _ensure_ntff_hook():
    """The agent image lacks ``antenv.axon_hooks``; shim it with the
    ctypes NTFF profiler from trn_agent_boot so trace=True works."""
    import types
    try:
        from antenv.axon_hooks import get_axon_ntff_profile_hook  # noqa
        return
    except ImportError:
        pass
    try:
        sys.path.insert(0, "/root/.axon_site")
        from trn_agent_boot.trn_boot import _ntff_profile_via_ctypes
        hook = _ntff_profile_via_ctypes("/opt/axon/libaxon_pjrt.so")
    except Exception:
        hook = None
    store = {"h": hook}
    pkg = sys.modules.setdefault("antenv", types.ModuleType("antenv"))
    mod = types.ModuleType("antenv.axon_hooks")
    mod.get_axon_ntff_profile_hook = lambda: store["h"]
    mod.set_axon_ntff_profile_hook = lambda h: store.update(h=h)
    pkg.axon_hooks = mod
    sys.modules["antenv.axon_hooks"] = mod


# revision 26
# speedup vs baseline: 1.6030x; 1.0116x over previous
"""Trainium2 Bass kernel for nn_EvacPolicy (segment_reduce).

Data-parallel over 8 NeuronCores: nodes sharded at graph boundaries, MLP
weights replicated, per-graph segment mean computed locally per shard,
heads computed locally per shard (row-wise independent), host concatenates
per-core outputs.

Segment mean strategy: graphs are sorted by (max-over-cores) size and packed
into runs of equal padded width s; every core places graph-at-position-p in
the SAME column range, so one SPMD program serves all 8 cores.  A single
DVE tensor_reduce over a [P, k, s] view then produces k graph sums at once.
The inf-branch stream stacks two graphs per column range (partitions 0:64 /
64:128), halving its reduce work.  All offsets are baked at trace time; the
program is rebuilt per kernel() call, nothing input-specific lives in this
file.

Engine budget: PE does the four per-node matmuls in bf16; ACT does the two
gelu layers fused with PSUM evacuation (optionally a few blocks' outer gelu
runs as an exact small-|u| quadratic on DVE to rebalance); DVE does the
grouped segment reduces.
"""

import math
import os
import sys
from contextlib import ExitStack

try:
    import concourse  # noqa: F401  (already on path, e.g. axon site)
except ImportError:
    for _p in ("/opt/trn_rl_repo",):
        if _p not in sys.path and os.path.isdir(_p):
            sys.path.insert(0, _p)

import numpy as np

import concourse.bass as bass
import concourse.bacc as bacc
import concourse.tile as tile
import concourse.mybir as mybir
from concourse.bass_utils import run_bass_kernel_spmd

FP32 = mybir.dt.float32
BF16 = mybir.dt.bfloat16
GELU = mybir.ActivationFunctionType.Gelu
IDENT = mybir.ActivationFunctionType.Identity
ADD = mybir.AluOpType.add
SUB = mybir.AluOpType.subtract
MUL = mybir.AluOpType.mult
AXX = mybir.AxisListType.X

N_CORES = 8
UNIT = 1024          # node columns per PSUM block (2 banks fp32)
MACRO = 4096         # node columns per SBUF feature tile (4 PSUM blocks)
MMN = 512            # max moving free dim per matmul into one PSUM bank
KMAX = 32            # max graphs per grouped reduce
GELU_C2 = 0.3989422804014327 / 2.0  # gelu(u) ~ u*(0.5 + GELU_C2*u), |u|<<1


def _round_up(x, m):
    return (x + m - 1) // m * m


def const_layout(G_PAD):
    return [
        ("b1a", 128, 1), ("b2a", 128, 1), ("b1i", 128, 1), ("b2i", 128, 1),
        ("pfa", 1, 128), ("pfb", 1, 64), ("ones", 1, 128),
        ("fc1w", 128, 256), ("fc1wb", 64, 256),
        ("fc1b0", 128, 1), ("fc1b1", 128, 1),
        ("fc2w0", 128, 128), ("fc2w1", 128, 128), ("fc2b", 128, 1),
        ("shgdw", 128, 2), ("shgdb", 2, 1),
        ("c1w", 128, 128), ("c1wb", 64, 128), ("c1b", 128, 1),
        ("c2w", 128, 64), ("c2b", 64, 1), ("c3w", 64, 1), ("c3b", 1, 1),
        ("recip", 1, G_PAD), ("npadA", 1, G_PAD), ("npadB", 1, G_PAD),
    ]


def pack_runs(widths, kmax=KMAX, macro=MACRO):
    """Pack (position-ordered, non-increasing) widths into runs.

    Returns (runs, col0, total_cols): runs = list of
    (macro_idx, src_off, k, s, pos0); col0[p] = start column of position p.
    """
    runs = []
    col0 = np.zeros(len(widths), np.int64)
    mac = 0
    mac_used = 0
    p = 0
    n = len(widths)
    while p < n:
        s = int(widths[p])
        assert s <= macro, f"graph width {s} exceeds macro tile {macro}"
        if mac_used + s > macro:
            mac += 1
            mac_used = 0
        k = 1
        while (p + k < n and k < kmax and widths[p + k] >= 0
               and mac_used + (k + 1) * s <= macro):
            k += 1
        for i in range(k):
            col0[p + i] = mac * macro + mac_used + i * s
        runs.append((mac, mac_used, k, s, p))
        mac_used += k * s
        p += k
    return runs, col0, (mac + 1) * macro


# ----------------------------------------------------------------------------
# device program
# ----------------------------------------------------------------------------

def build_program(NCA, NHB, G_PAD, GS, runs_a, runs_b, n_poly=0):
    nc = bacc.Bacc("TRN2", target_bir_lowering=False, debug=False,
                   num_devices=N_CORES)

    layout = const_layout(G_PAD)
    WCOLS = sum(c for _, _, c in layout)

    xph = nc.dram_tensor("xph", [8, NCA], BF16, kind="ExternalInput")
    xinf = nc.dram_tensor("xinf", [6, NHB], BF16, kind="ExternalInput")
    wblob_d = nc.dram_tensor("wblob", [128, WCOLS], FP32,
                             kind="ExternalInput")
    wb16_d = nc.dram_tensor("wb16", [128, 512], BF16, kind="ExternalInput")
    o_sg = nc.dram_tensor("o_sg", [2, G_PAD], FP32, kind="ExternalOutput")
    o_v = nc.dram_tensor("o_v", [1, G_PAD], FP32, kind="ExternalOutput")

    NMA = NCA // MACRO
    NMB = NHB // MACRO
    runs_a_by_mac = [[] for _ in range(NMA)]
    for mac, off, k, s, pos0 in runs_a:
        runs_a_by_mac[mac].append((off, k, s, pos0))
    runs_b_by_mac = [[] for _ in range(NMB)]
    for mac, off, k, s, pos0 in runs_b:
        runs_b_by_mac[mac].append((off, k, s, pos0))

    with tile.TileContext(nc) as tc, ExitStack() as ctx:
        const = ctx.enter_context(tc.tile_pool(name="const", bufs=1))
        stage = ctx.enter_context(tc.tile_pool(name="stage", bufs=1))

        wblob = const.tile([128, WCOLS], FP32, name="wblob")
        nc.sync.dma_start(wblob[:, :], wblob_d[:, :])
        W = {}
        off = 0
        for name, rows, cols in layout:
            W[name] = wblob[0:rows, off:off + cols]
            off += cols
        wb16 = const.tile([128, 512], BF16, name="wb16")
        nc.sync.dma_start(wb16[:, :], wb16_d[:, :])
        W["w1a"] = wb16[0:8, 0:128]
        W["w2ph"] = wb16[0:128, 128:256]
        W["w1i"] = wb16[0:6, 256:384]
        W["w2i"] = wb16[0:128, 384:512]

        suma = stage.tile([128, G_PAD], FP32, name="suma")
        sumbr = stage.tile([128, GS], FP32, name="sumbr")
        sumb = stage.tile([64, G_PAD], FP32, name="sumb")

        tc.strict_bb_all_engine_barrier()

        # ---------------- node pipeline ----------------
        with tc.tile_pool(name="xin", bufs=3) as xin, \
             tc.tile_pool(name="mid", bufs=3) as mid, \
             tc.tile_pool(name="scn", bufs=2) as scn, \
             tc.tile_pool(name="poly", bufs=2) as ptmp, \
             tc.tile_pool(name="psum", bufs=2, space="PSUM") as psum:

            poly_left = [n_poly]

            def outer_evac(ps2, sa_t, c0, bias, branch):
                """outer gelu PSUM block -> sa_t[:, c0:c0+UNIT]"""
                dst = sa_t[:, c0:c0 + UNIT]
                if poly_left[0] > 0:
                    poly_left[0] -= 1
                    t1 = ptmp.tile([128, UNIT], FP32, name="pt", tag="pt")
                    nc.vector.tensor_scalar(t1[:, :], ps2[:, :], GELU_C2,
                                            0.5, MUL, ADD)
                    nc.vector.tensor_tensor(dst, t1[:, :], ps2[:, :], MUL)
                else:
                    nc.scalar.activation(dst, ps2[:, :], GELU, bias=bias)

            def mlp_block(xt_src, u, w1, w2, b1, b2, sa_t, c0, tagp):
                """one UNIT of nodes: x -> gelu -> L2 -> outer -> sa_t"""
                xt = xin.tile(list(xt_src[0]), BF16, name="xt" + tagp,
                              tag="xt" + tagp)
                nc.sync.dma_start(xt[:, :], xt_src[1])
                ps1 = psum.tile([128, UNIT], FP32, name="p1" + tagp,
                                tag="l1")
                for kk in range(UNIT // MMN):
                    s = slice(kk * MMN, (kk + 1) * MMN)
                    nc.tensor.matmul(ps1[:, s], w1, xt[:, s],
                                     start=True, stop=True)
                h1 = mid.tile([128, UNIT], BF16, name="h1" + tagp,
                              tag="h1" + tagp)
                nc.scalar.activation(h1[:, :], ps1[:, :], GELU, bias=b1)
                ps2 = psum.tile([128, UNIT], FP32, name="p2" + tagp,
                                tag="l2")
                for kk in range(UNIT // MMN):
                    s = slice(kk * MMN, (kk + 1) * MMN)
                    nc.tensor.matmul(ps2[:, s], w2, h1[:, s],
                                     start=True, stop=True)
                outer_evac(ps2, sa_t, c0, b2, tagp)

            def reduce_macro(sa_t, rlist, out_t, out_map):
                for off, k, s, pos0 in rlist:
                    src = sa_t[:, off:off + k * s]
                    if k > 1:
                        src = src.rearrange("p (k s) -> p k s", k=k)
                    else:
                        src = sa_t[:, off:off + s].rearrange(
                            "p (k s) -> p k s", k=1)
                    o0 = out_map(pos0)
                    nc.vector.tensor_reduce(out_t[:, o0:o0 + k], src,
                                            AXX, ADD)

            bi = 0
            sa_tiles = {}

            def a_macro(m):
                sa_t = scn.tile([128, MACRO], FP32, name="saA", tag="saA")
                for j in range(MACRO // UNIT):
                    u = m * (MACRO // UNIT) + j
                    c = u * UNIT
                    mlp_block(([8, UNIT], xph[:, c:c + UNIT]), u,
                              W["w1a"], W["w2ph"], W["b1a"], W["b2a"],
                              sa_t, j * UNIT, "A")
                reduce_macro(sa_t, runs_a_by_mac[m], suma, lambda p: p)

            def b_macro(m):
                sa_t = scn.tile([128, MACRO], FP32, name="saB", tag="saB")
                for j in range(MACRO // UNIT):
                    u = m * (MACRO // UNIT) + j
                    c = u * UNIT
                    mlp_block(([6, UNIT], xinf[:, c:c + UNIT]), u,
                              W["w1i"], W["w2i"], W["b1i"], W["b2i"],
                              sa_t, j * UNIT, "B")
                reduce_macro(sa_t, runs_b_by_mac[m], sumbr, lambda p: p)

            bi = 0
            for m in range(NMA):
                a_macro(m)
                if m % 2 == 1 and bi < NMB:
                    b_macro(bi)
                    bi += 1
            while bi < NMB:
                b_macro(bi)
                bi += 1

        # ---------------- assemble per-graph sums ----------------
        tc.strict_bb_all_engine_barrier()
        # half-0 sums sit on partitions 0:64 at positions 0..GS-1
        nc.vector.tensor_copy(sumb[:, 0:GS], sumbr[0:64, :])
        # half-1 sums: partitions 64:128 -> partition shift via DMA
        nc.sync.dma_start(sumb[:, GS:G_PAD], sumbr[64:128, 0:G_PAD - GS])

        # ---------------- heads ----------------
        tc.strict_bb_all_engine_barrier()
        with tc.tile_pool(name="hps", bufs=2, space="PSUM") as hps, \
             tc.tile_pool(name="hsb", bufs=2) as hsb:

            for g0 in range(0, G_PAD, MMN):
                gs = min(MMN, G_PAD - g0)
                gsl = slice(g0, g0 + gs)

                pr = hps.tile([128, gs], FP32, name="pr", tag="hp")
                nc.tensor.matmul(pr[:, :], W["ones"], W["recip"][:, gsl],
                                 start=True, stop=True)
                pc = hps.tile([128, gs], FP32, name="pc", tag="hp")
                nc.tensor.matmul(pc[:, :], W["pfa"], W["npadA"][:, gsl],
                                 start=True, stop=True)

                gea = hsb.tile([128, gs], FP32, name="gea", tag="ga")
                nc.vector.tensor_tensor(gea[:, :], suma[:, gsl], pc[:, :],
                                        SUB)
                nc.vector.tensor_tensor(gea[:, :], gea[:, :], pr[:, :], MUL)

                pcb = hps.tile([64, gs], FP32, name="pcb", tag="hp")
                nc.tensor.matmul(pcb[:, :], W["pfb"], W["npadB"][:, gsl],
                                 start=True, stop=True)
                geb = hsb.tile([64, gs], FP32, name="geb", tag="gb")
                nc.vector.tensor_tensor(geb[:, :], sumb[:, gsl], pcb[:, :],
                                        SUB)
                nc.vector.tensor_tensor(geb[:, :], geb[:, :], pr[0:64, :],
                                        MUL)

                def lin2(lhs_a, lhs_b, bias_t, act, m0, m1, name):
                    p = hps.tile([m1 - m0, gs], FP32, name="p" + name,
                                 tag="hp")
                    nc.tensor.matmul(p[:, :], lhs_a[:, m0:m1], gea[:, :],
                                     start=True, stop=False)
                    nc.tensor.matmul(p[:, :], lhs_b[:, m0:m1], geb[:, :],
                                     start=False, stop=True)
                    o = hsb.tile([m1 - m0, gs], FP32, name="s" + name,
                                 tag="hs" + name)
                    nc.scalar.activation(o[:, :], p[:, :], act, bias=bias_t)
                    return o

                d10 = lin2(W["fc1w"], W["fc1wb"], W["fc1b0"], GELU,
                           0, 128, "d10")
                d11 = lin2(W["fc1w"], W["fc1wb"], W["fc1b1"], GELU,
                           128, 256, "d11")

                pd2 = hps.tile([128, gs], FP32, name="pd2", tag="hp")
                nc.tensor.matmul(pd2[:, :], W["fc2w0"], d10[:, :],
                                 start=True, stop=False)
                nc.tensor.matmul(pd2[:, :], W["fc2w1"], d11[:, :],
                                 start=False, stop=True)
                d2 = hsb.tile([128, gs], FP32, name="d2", tag="d2")
                nc.scalar.activation(d2[:, :], pd2[:, :], GELU,
                                     bias=W["fc2b"])

                psg = hps.tile([2, gs], FP32, name="psg", tag="hp")
                nc.tensor.matmul(psg[:, :], W["shgdw"], d2[:, :],
                                 start=True, stop=True)
                sg = hsb.tile([2, gs], FP32, name="sg", tag="sg")
                nc.scalar.activation(sg[:, :], psg[:, :], IDENT,
                                     bias=W["shgdb"])
                nc.sync.dma_start(o_sg[:, gsl], sg[:, :])

                v1 = lin2(W["c1w"], W["c1wb"], W["c1b"], GELU, 0, 128, "v1")
                pv2 = hps.tile([64, gs], FP32, name="pv2", tag="hp")
                nc.tensor.matmul(pv2[:, :], W["c2w"], v1[:, :],
                                 start=True, stop=True)
                v2 = hsb.tile([64, gs], FP32, name="v2", tag="v2")
                nc.scalar.activation(v2[:, :], pv2[:, :], GELU,
                                     bias=W["c2b"])
                pv3 = hps.tile([1, gs], FP32, name="pv3", tag="hp")
                nc.tensor.matmul(pv3[:, :], W["c3w"], v2[:, :],
                                 start=True, stop=True)
                vo = hsb.tile([1, gs], FP32, name="vo", tag="vo")
                nc.scalar.activation(vo[:, :], pv3[:, :], IDENT,
                                     bias=W["c3b"])
                nc.sync.dma_start(o_v[:, gsl], vo[:, :])

    return nc


# ----------------------------------------------------------------------------
# host wrapper
# ----------------------------------------------------------------------------

_cache = {}


def _np_gelu(x):
    v = np.vectorize(math.erf)
    return 0.5 * x * (1.0 + v(x / math.sqrt(2.0)))


def _blockdiag(blocks, K, M):
    out = np.zeros((K, M), np.float32)
    for Wm, r, c in blocks:
        out[r:r + Wm.shape[0], c:c + Wm.shape[1]] = Wm
    return out


N_POLY = int(os.environ.get("KERNEL_N_POLY", "0"))


def kernel(x_ped, x_hazard, x_infra, batch, num_graphs,
           ped_W1, ped_b1, ped_W2, ped_b2,
           haz_W1, haz_b1, haz_W2, haz_b2,
           inf_W1, inf_b1, inf_W2, inf_b2,
           fc1_W, fc1_b, fc2_W, fc2_b,
           sh_W, sh_b, gd_W, gd_b,
           c1_W, c1_b, c2_W, c2_b, c3_W, c3_b):
    import ml_dtypes
    bf16 = ml_dtypes.bfloat16

    x_ped = np.asarray(x_ped, np.float32)
    x_hazard = np.asarray(x_hazard, np.float32)
    x_infra = np.asarray(x_infra, np.float32)
    batch = np.asarray(batch).astype(np.int64)
    B = int(num_graphs)
    N = batch.shape[0]

    f32 = lambda a: np.ascontiguousarray(np.asarray(a), dtype=np.float32)
    (ped_W1, ped_b1, ped_W2, ped_b2, haz_W1, haz_b1, haz_W2, haz_b2,
     inf_W1, inf_b1, inf_W2, inf_b2, fc1_W, fc1_b, fc2_W, fc2_b,
     sh_W, sh_b, gd_W, gd_b, c1_W, c1_b, c2_W, c2_b, c3_W, c3_b) = map(
        f32, (ped_W1, ped_b1, ped_W2, ped_b2, haz_W1, haz_b1, haz_W2,
              haz_b2, inf_W1, inf_b1, inf_W2, inf_b2, fc1_W, fc1_b, fc2_W,
              fc2_b, sh_W, sh_b, gd_W, gd_b, c1_W, c1_b, c2_W, c2_b, c3_W,
              c3_b))

    # ---- shard graphs across cores, balancing node counts ----
    ends = np.searchsorted(batch, np.arange(B), side="right")
    gsplits = [0]
    for c in range(1, N_CORES):
        gsplits.append(int(np.searchsorted(ends, N * c // N_CORES)))
    gsplits.append(B)
    g_lo = np.array(gsplits[:-1])
    g_hi = np.maximum(np.array(gsplits[1:]), g_lo)

    counts_all = np.diff(np.concatenate([[0], ends])).astype(np.int64)
    G_PAD = _round_up(max(int((g_hi - g_lo).max()), 2), 64)

    cnt = np.zeros((N_CORES, G_PAD), np.int64)
    for c in range(N_CORES):
        g = g_hi[c] - g_lo[c]
        cnt[c, :g] = counts_all[g_lo[c]:g_hi[c]]
    maxcnt = np.maximum(cnt.max(axis=0), 1)

    # ---- stream positions: rank graphs by width, interleave halves ----
    order = np.argsort(-maxcnt, kind="stable")       # local idx by rank
    GS = (G_PAD + 1) // 2
    pos_of = np.zeros(G_PAD, np.int64)               # local idx -> position
    for r, j in enumerate(order):
        q, h = divmod(r, 2)
        pos_of[j] = q if h == 0 else GS + q
    # guard: ranks 2q/2q+1 -> positions q / GS+q; q < GS always
    idx_at = np.zeros(G_PAD, np.int64)               # position -> local idx
    idx_at[pos_of] = np.arange(G_PAD)

    wA = maxcnt[idx_at]                              # width by position
    # A-stream: positions 0..GS-1 then GS..: widths non-increasing within
    # each half (sorted interleave), pack runs per half consecutively
    runs_a0, col0_a0, colsA0 = pack_runs(wA[:GS])
    runs_a1, col0_a1, colsA1 = pack_runs(wA[GS:])
    off_macs = colsA0 // MACRO
    runs_a = runs_a0 + [(m + off_macs, o, k, s, p + GS)
                        for m, o, k, s, p in runs_a1]
    col0A = np.concatenate([col0_a0, col0_a1 + colsA0])
    NCA = colsA0 + colsA1

    # B-stream: pair (pos q, pos GS+q) share a column range
    wB = np.maximum(wA[:GS], np.concatenate(
        [wA[GS:], np.ones(2 * GS - G_PAD, np.int64)]))
    runs_b, col0B_pair, NHB = pack_runs(wB)
    col0B = np.concatenate([col0B_pair, col0B_pair[:G_PAD - GS]])

    # ---- per-core node arrays ----
    starts_all = ends - counts_all
    x_all = np.concatenate([x_ped, x_hazard, x_infra], axis=1)

    in_maps = []
    for c in range(N_CORES):
        n0 = int(ends[g_lo[c] - 1]) if g_lo[c] > 0 else 0
        n1 = int(ends[g_hi[c] - 1]) if g_hi[c] > 0 else 0
        ncr = n1 - n0
        g = batch[n0:n1] - g_lo[c]                   # local graph idx
        within = np.arange(ncr) - (starts_all[batch[n0:n1]] - n0)
        p = pos_of[g]
        destA = col0A[p] + within
        destB = col0B[p] + within
        hB = (p >= GS).astype(np.int64)

        xph_a = np.zeros((8, NCA), bf16)
        xph_a[:, destA] = x_all[n0:n1].T.astype(bf16)
        xinf_a = np.zeros((6, NHB), bf16)
        xinf_T = x_infra[n0:n1].T.astype(bf16)
        lo = hB == 0
        xinf_a[0:3, destB[lo]] = xinf_T[:, lo]
        xinf_a[3:6, destB[~lo]] = xinf_T[:, ~lo]
        in_maps.append({"xph": xph_a, "xinf": xinf_a})

    # feature vector produced by zero-input (padding) nodes
    pf = []
    for W1, b1, W2, b2 in ((ped_W1, ped_b1, ped_W2, ped_b2),
                           (haz_W1, haz_b1, haz_W2, haz_b2),
                           (inf_W1, inf_b1, inf_W2, inf_b2)):
        pf.append(_np_gelu(_np_gelu(b1) @ W2 + b2))
    pad_feat = np.concatenate(pf).astype(np.float32)

    consts = {
        "b1a": np.concatenate([ped_b1, haz_b1]).reshape(128, 1),
        "b2a": np.concatenate([ped_b2, haz_b2]).reshape(128, 1),
        "b1i": np.concatenate([inf_b1, inf_b1]).reshape(128, 1),
        "b2i": np.concatenate([inf_b2, inf_b2]).reshape(128, 1),
        "pfa": pad_feat[:128].reshape(1, 128),
        "pfb": pad_feat[128:].reshape(1, 64),
        "ones": np.ones((1, 128), np.float32),
        "fc1w": fc1_W[0:128], "fc1wb": fc1_W[128:192],
        "fc1b0": fc1_b[0:128].reshape(128, 1),
        "fc1b1": fc1_b[128:256].reshape(128, 1),
        "fc2w0": fc2_W[0:128], "fc2w1": fc2_W[128:256],
        "fc2b": fc2_b.reshape(128, 1),
        "shgdw": np.concatenate([sh_W, gd_W], axis=1),
        "shgdb": np.array([[float(sh_b.ravel()[0])],
                           [float(gd_b.ravel()[0])]], np.float32),
        "c1w": c1_W[0:128], "c1wb": c1_W[128:192],
        "c1b": c1_b.reshape(128, 1),
        "c2w": c2_W, "c2b": c2_b.reshape(64, 1),
        "c3w": c3_W, "c3b": c3_b.reshape(1, 1),
    }

    layout = const_layout(G_PAD)
    WCOLS = sum(c for _, _, c in layout)
    blob_common = np.zeros((128, WCOLS), np.float32)
    slices = {}
    off = 0
    for name, rows, cols in layout:
        slices[name] = (rows, slice(off, off + cols))
        if name in consts:
            blob_common[0:rows, off:off + cols] = consts[name]
        off += cols

    wb16 = np.zeros((128, 512), bf16)
    wb16[0:8, 0:128] = _blockdiag(
        [(ped_W1, 0, 0), (haz_W1, 2, 64)], 8, 128).astype(bf16)
    wb16[0:128, 128:256] = _blockdiag(
        [(ped_W2, 0, 0), (haz_W2, 64, 64)], 128, 128).astype(bf16)
    wb16[0:6, 256:384] = _blockdiag(
        [(inf_W1, 0, 0), (inf_W1, 3, 64)], 6, 128).astype(bf16)
    wb16[0:128, 384:512] = _blockdiag(
        [(inf_W2, 0, 0), (inf_W2, 64, 64)], 128, 128).astype(bf16)

    cnt_pos = cnt[:, idx_at]                         # (cores, positions)
    sA = wA                                          # padded width (A)
    sB = np.concatenate([wB, wB[:G_PAD - GS]])       # padded width (B)
    for c in range(N_CORES):
        blob = blob_common.copy()
        rows, sl = slices["recip"]
        blob[0:rows, sl] = (1.0 / np.maximum(cnt_pos[c], 1)).astype(
            np.float32)
        rows, sl = slices["npadA"]
        blob[0:rows, sl] = (sA - cnt_pos[c]).astype(np.float32)
        rows, sl = slices["npadB"]
        blob[0:rows, sl] = (sB - cnt_pos[c]).astype(np.float32)
        in_maps[c]["wblob"] = blob
        in_maps[c]["wb16"] = wb16

    # ---- build / fetch program ----
    # the quadratic outer-gelu path assumes zero second-layer biases
    n_poly = N_POLY
    if (np.any(ped_b2) or np.any(haz_b2) or np.any(inf_b2)):
        n_poly = 0
    key = (NCA, NHB, G_PAD, GS, tuple(map(tuple, runs_a)),
           tuple(map(tuple, runs_b)), n_poly)
    if key not in _cache:
        _cache.clear()
        nc_new = build_program(NCA, NHB, G_PAD, GS, runs_a, runs_b,
                               n_poly=n_poly)
        if not nc_new.is_finalized():
            nc_new.finalize()
        _cache[key] = nc_new
    nc = _cache[key]

    trace = bool(os.environ.get("BASS_PROFILE"))
    if trace:
        _ensure_ntff_hook()
    res = run_bass_kernel_spmd(nc, in_maps, list(range(N_CORES)),
                               trace=trace)
    if trace and res.exec_time_ns is not None:
        kernel.last_exec_time_ns = res.exec_time_ns
        kernel.last_result = res
        print(f"HW exec time: {res.exec_time_ns} ns")

    # ---- assemble full outputs (positions -> graph ids) ----
    shelter = np.zeros((B, 1), np.float32)
    guidance = np.zeros((B, 1), np.float32)
    value = np.zeros((B,), np.float32)
    for c in range(N_CORES):
        g = g_hi[c] - g_lo[c]
        if g == 0:
            continue
        out_sg = res.results[c]["o_sg"]
        out_v = res.results[c]["o_v"]
        pg = pos_of[np.arange(g)]
        shelter[g_lo[c]:g_hi[c], 0] = out_sg[0, pg]
        guidance[g_lo[c]:g_hi[c], 0] = out_sg[1, pg]
        value[g_lo[c]:g_hi[c]] = out_v[0, pg]
    return shelter, guidance, value


kernel.last_exec_time_ns = None
kernel.last_result = None


def _ensure_ntff_hook():
    """The agent image lacks ``antenv.axon_hooks``; shim it with the
    ctypes NTFF profiler from trn_agent_boot so trace=True works."""
    import types
    try:
        from antenv.axon_hooks import get_axon_ntff_profile_hook  # noqa
        return
    except ImportError:
        pass
    try:
        sys.path.insert(0, "/root/.axon_site")
        from trn_agent_boot.trn_boot import _ntff_profile_via_ctypes
        hook = _ntff_profile_via_ctypes("/opt/axon/libaxon_pjrt.so")
    except Exception:
        hook = None
    store = {"h": hook}
    pkg = sys.modules.setdefault("antenv", types.ModuleType("antenv"))
    mod = types.ModuleType("antenv.axon_hooks")
    mod.get_axon_ntff_profile_hook = lambda: store["h"]
    mod.set_axon_ntff_profile_hook = lambda h: store.update(h=h)
    pkg.axon_hooks = mod
    sys.modules["antenv.axon_hooks"] = mod
